# revision 35
# baseline (speedup 1.0000x reference)
"""Trainium2 Bass kernel for nn_BandFunctionalPrior.

Math reduction: e = x*W_e + b_e is affine in the scalar x, so the
attention scores collapse to rank-1 plus j-constant terms that cancel
in softmax:

    s[t,n,i,j] = u_i * x_j  (+ j-const, cancelled)
    u = (alpha_n * x + gamma_n) / sqrt(d_s)

Per (batch, t, band): P = row-softmax(u x^T + maskbias), output
P_func = sum_n w_n P_n plus a tiny embeddings path.  Pure data
parallel over batch (4 per core, 8 cores).

Device geometry: for each (b, band), 128 t-values form 8 tile groups
of 16; a tile group's 16 stacked 64x64 score matrices are computed by
ONE K=112 bf16 matmul into a [128, 512] PSUM slice:
  rows  0..16: uh pair-diagonal (block h-diag of bf16-high u)
  rows 16..32: ul pair-diagonal (bf16 residual of u)
  rows 32..48: uh pair-diagonal again
  rows 48..112: [I64|I64] selection rows (constant)
against rhs rows (xh-diag, xh-diag, xl-diag, mb8) — i.e. the split
product uh*xh + ul*xh + uh*xl (abs err ~|s|*2^-18) with the adjacency
mask bias added via the constant rows.  ACT exponentiates 4 tiles at a
time ([128, 2048] PSUM -> bf16 SBUF); DVE does row sums + reciprocal +
weighting + the final fp32 combine; GPSIMD expands the row scales and
does the intermediate band accumulations.
"""

import sys

for _p in ("/opt/trn_rl_repo",):
    if _p not in sys.path:
        sys.path.insert(0, _p)

import ml_dtypes
import numpy as np

BF16 = np.dtype(ml_dtypes.bfloat16)

NCORES = 8
B4 = 4          # batches per core
NB = 5          # bands
C = 64          # electrodes
T = 128         # time steps
TG = 8          # tile groups per (b, band); 16 t's each
NSB = 2         # supertiles per (b, band); 4 tile groups each
NEG = -30000.0  # mask bias; exp(s + NEG) == 0 in fp32

_CACHE = {}


def _build_program():
    import concourse.bacc as bacc
    import concourse.mybir as mybir
    from concourse.tile import TileContext

    fp32 = mybir.dt.float32
    bf16 = mybir.dt.bfloat16
    ALU = mybir.AluOpType
    ACTF = mybir.ActivationFunctionType
    AXIS = mybir.AxisListType

    nc = bacc.Bacc(None, target_bir_lowering=False)

    ukb = nc.dram_tensor("ukb", (B4, NB, 48, TG * 128), bf16, kind="ExternalInput")
    xkb = nc.dram_tensor("xkb", (B4, NB, 48, TG * 512), bf16, kind="ExternalInput")
    xo = nc.dram_tensor("xo", (B4, NB, C, T), fp32, kind="ExternalInput")
    wbig = nc.dram_tensor("wbig", (T, B4 * NB * C), fp32, kind="ExternalInput")
    md = nc.dram_tensor("md", (B4, T, NB), fp32, kind="ExternalInput")
    mb8 = nc.dram_tensor("mb8", (C, TG * C), bf16, kind="ExternalInput")
    i2 = nc.dram_tensor("i2", (C, 2 * C), bf16, kind="ExternalInput")
    wemb = nc.dram_tensor("wemb", (2 * NB, C), fp32, kind="ExternalInput")
    ones64 = nc.dram_tensor("ones64", (C, 1), fp32, kind="ExternalInput")
    id128 = nc.dram_tensor("id128", (T, T), fp32, kind="ExternalInput")
    pfunc = nc.dram_tensor("pfunc", (B4, T, C, C), fp32, kind="ExternalOutput")
    emb = nc.dram_tensor("emb", (B4, T, C), fp32, kind="ExternalOutput")

    SB = 4 * 512          # supertile free width (4 tile groups)
    KK = 112              # matmul contraction: 16*3 pair rows + 64 mask rows

    with TileContext(nc) as tc:
        # ---------------- embeddings (tiny, own psum scope) ----------------
        with (
            tc.tile_pool(name="econst", bufs=1) as ecp,
            tc.tile_pool(name="ework", bufs=4) as ewp,
            tc.tile_pool(name="epsum", bufs=2, space="PSUM") as epp,
        ):
            wemb_sb = ecp.tile([2 * NB, C], fp32)
            nc.sync.dma_start(wemb_sb, wemb[:, :])
            ones_sb = ecp.tile([C, 1], fp32)
            nc.sync.dma_start(ones_sb, ones64[:, :])
            id128_sb = ecp.tile([T, T], fp32)
            nc.sync.dma_start(id128_sb, id128[:, :])
            for b in range(B4):
                xbars = epp.tile([T, NB], fp32, tag="xbars")
                for n in range(NB):
                    xo_sb = ewp.tile([C, T], fp32, tag="xo")
                    nc.sync.dma_start(xo_sb, xo[b, n])
                    nc.tensor.matmul(
                        xbars[:, n : n + 1], xo_sb, ones_sb, start=True, stop=True
                    )
                mdv_sb = ewp.tile([T, NB], fp32, tag="mdv")
                nc.sync.dma_start(mdv_sb, md[b])
                coef = ewp.tile([T, 2 * NB], fp32, tag="coef")
                nc.vector.tensor_mul(coef[:, :NB], xbars[:, :], mdv_sb[:, :])
                nc.vector.tensor_copy(coef[:, NB:], mdv_sb[:, :])
                coefT_ps = epp.tile([2 * NB, T], fp32, tag="coefT")
                nc.tensor.transpose(coefT_ps, coef, id128_sb)
                coefT_sb = ewp.tile([2 * NB, T], fp32, tag="coefTs")
                nc.vector.tensor_copy(coefT_sb, coefT_ps)
                emb_ps = epp.tile([T, C], fp32, tag="embp")
                nc.tensor.matmul(emb_ps, coefT_sb, wemb_sb, start=True, stop=True)
                emb_sb = ewp.tile([T, C], fp32, tag="embs")
                nc.vector.tensor_copy(emb_sb, emb_ps)
                nc.sync.dma_start(emb[b], emb_sb)

        # ---------------- attention ----------------
        with (
            tc.tile_pool(name="banks", bufs=1) as bankp,
            tc.tile_pool(name="fwork", bufs=3) as fworkp,
            tc.tile_pool(name="paccp", bufs=2) as paccp,
            tc.tile_pool(name="pout", bufs=2) as poutp,
            tc.tile_pool(name="small", bufs=8) as smallp,
            tc.tile_pool(name="psum", bufs=2, space="PSUM") as psump,
        ):
            # persistent banks (ping-pong x2): const mask rows written once,
            # u/x rows DMA'd per (b, n) from host-prebuilt images
            kbanks, xkbanks = [], []
            for kk in range(2):
                kbk = bankp.tile([KK, TG * 128], bf16, name=f"kbank{kk}")
                for tgb in range(TG):
                    i2_dst = kbk[48:, tgb * 128 : (tgb + 1) * 128]
                    nc.sync.dma_start(i2_dst, i2[:, :])
                kbanks.append(kbk)
                xbk = bankp.tile([KK, TG * 512], bf16, name=f"xkbank{kk}")
                for tgb in range(TG):
                    mb_dst = xbk[48:, tgb * 512 : (tgb + 1) * 512]
                    nc.sync.dma_start(mb_dst, mb8[:, :])
                xkbanks.append(xbk)
            # full w table, loaded once: [128=(h,i), (b, n, tg, m)]
            wt = bankp.tile([T, B4 * NB * C], fp32, name="wtall")
            nc.sync.dma_start(wt, wbig[:, :])

            for b in range(B4):
                pacc = [
                    paccp.tile([128, 2 * SB], bf16, tag="pacc", name=f"pacc_{b}")
                ]
                for n in range(NB):
                    kk = (b * NB + n) % 2
                    kbk, xbk = kbanks[kk], xkbanks[kk]
                    # fill u/x rows from host-prebuilt images: 2 big DMAs
                    nc.sync.dma_start(kbk[:48, :], ukb[b, n])
                    nc.sync.dma_start(xbk[:48, :], xkb[b, n])
                    wcol0 = (b * NB + n) * C

                    F = fworkp.tile([128, 2 * SB], bf16, tag="F")
                    for half in range(2):
                        ps = psump.tile([128, SB], fp32, tag="ps")
                        for q2 in range(4):
                            tg = half * 4 + q2
                            nc.tensor.matmul(
                                ps[:, q2 * 512 : (q2 + 1) * 512],
                                kbk[:, tg * 128 : (tg + 1) * 128],
                                xbk[:, tg * 512 : (tg + 1) * 512],
                                start=True,
                                stop=True,
                            )
                        nc.scalar.activation(
                            F[:, half * SB : (half + 1) * SB], ps, ACTF.Exp
                        )
                    Fv = F.rearrange("p (m j) -> p m j", j=C)
                    Z = smallp.tile([128, 8 * TG], fp32, tag="Z")
                    nc.vector.tensor_reduce(Z, Fv, axis=AXIS.X, op=ALU.add)
                    rr = smallp.tile([128, 8 * TG], fp32, tag="rr")
                    nc.vector.reciprocal(rr, Z)
                    r2 = smallp.tile([128, 8 * TG], bf16, tag="r2")
                    nc.vector.tensor_mul(r2, rr, wt[:, wcol0 : wcol0 + C])
                    rb = r2[:, :, None].to_broadcast((128, 8 * TG, C))
                    if n == 0:
                        nc.vector.tensor_mul(
                            pacc[0].rearrange("p (m j) -> p m j", j=C), rb, Fv)
                    elif n < NB - 1:
                        tmp = fworkp.tile([128, 2 * SB], bf16, tag="tmp")
                        nc.vector.tensor_mul(
                            tmp.rearrange("p (m j) -> p m j", j=C), rb, Fv)
                        if n <= 2:
                            nc.gpsimd.tensor_add(pacc[0], pacc[0], tmp)
                        else:
                            nc.vector.tensor_add(pacc[0], pacc[0], tmp)
                    else:
                        tmp = fworkp.tile([128, 2 * SB], bf16, tag="tmp")
                        nc.vector.tensor_mul(
                            tmp.rearrange("p (m j) -> p m j", j=C), rb, Fv)
                        po = poutp.tile([128, 2 * SB], fp32, tag="po")
                        nc.vector.tensor_add(po, pacc[0], tmp)
                        # scatter to pfunc[b, t, i, j], t = tg*16+2m+h;
                        # one DMA per h, streaming (i, tg, m, j)
                        dview = pfunc[b].rearrange(
                            "(tg m2 h) i j -> h i tg m2 j", tg=TG, m2=8, h=2
                        )
                        for h in range(2):
                            sview = po[h * C : (h + 1) * C, :].rearrange(
                                "i (tg m j) -> i tg m j", tg=TG, j=C
                            )
                            nc.sync.dma_start(dview[h], sview)
    nc.finalize()
    return nc


def _get_program():
    if "nc" not in _CACHE:
        _CACHE["nc"] = _build_program()
    return _CACHE["nc"]


def _split_bf16(a):
    hi = a.astype(BF16)
    lo = (a - hi.astype(np.float32)).astype(BF16)
    return hi, lo


def kernel(x, adj, active_mask, active_alpha, W_e, b_e, Wq, Wk):
    x = np.asarray(x, np.float32)
    adj = np.asarray(adj, np.float32)
    active_mask = np.asarray(active_mask, np.float32)
    active_alpha = np.asarray(active_alpha, np.float32)
    W_e = np.asarray(W_e, np.float32)
    b_e = np.asarray(b_e, np.float32)
    Wq = np.asarray(Wq, np.float32)
    Wk = np.asarray(Wk, np.float32)

    ds = Wq.shape[0]
    Aq = W_e @ Wq.T
    Ak = W_e @ Wk.T
    Bq = b_e @ Wq.T
    a = (Aq * Ak).sum(-1) / np.sqrt(ds)
    g = (Bq * Ak).sum(-1) / np.sqrt(ds)

    xt = np.ascontiguousarray(x.transpose(0, 1, 3, 2))          # (32, 5, 128, 64)
    utf = (a[None, :, None, None] * xt + g[None, :, None, None]).astype(np.float32)
    xh, xl = _split_bf16(xt)
    uh, ul = _split_bf16(utf)

    # bank images; rows (g, m, h) = 16g + 2m + h
    # ukb cols (tg, c2, i): u_g(t=tg*16+2m+h) at c2 == h
    # xkb cols (tg, mc, j): x_g(t) at mc == m
    ukb = np.zeros((32, NB, 48, TG * 128), BF16)
    xkb = np.zeros((32, NB, 48, TG * 512), BF16)
    ukb_v = ukb.reshape(32, NB, 3, 8, 2, TG, 2, C)
    xkb_v = xkb.reshape(32, NB, 3, 8, 2, TG, 8, C)
    for gi, us in ((0, uh), (1, ul), (2, uh)):
        uu = us.reshape(32, NB, TG, 8, 2, C)                     # (b,n,tg,m,h,i)
        for h in range(2):
            ukb_v[:, :, gi, :, h, :, h, :] = uu[:, :, :, :, h, :].transpose(
                0, 1, 3, 2, 4
            )
    for gi, xs in ((0, xh), (1, xh), (2, xl)):
        xx = xs.reshape(32, NB, TG, 8, 2, C)
        for h in range(2):
            for m in range(8):
                xkb_v[:, :, gi, m, h, :, m, :] = xx[:, :, :, m, h, :]

    w = (active_alpha * active_mask).astype(np.float32)          # (32, 128, 5)
    cnt = np.clip(active_mask.sum(-1, keepdims=True), 1.0, None)
    mdv = (active_mask / cnt).astype(np.float32)                 # (32, 128, 5)
    mb = np.where(adj > 0, 0.0, NEG).astype(np.float32)
    mb8_np = np.ascontiguousarray(np.tile(mb, (1, TG))).astype(BF16)
    i2_np = np.ascontiguousarray(
        np.concatenate([np.eye(C, dtype=np.float32)] * 2, axis=1)
    ).astype(BF16)
    wemb_np = np.ascontiguousarray(
        np.concatenate([W_e / float(C), b_e], axis=0)
    ).astype(np.float32)
    ones_np = np.ones((C, 1), np.float32)
    id128_np = np.eye(T, dtype=np.float32)

    # w rearranged for the (h, i)-partition tiles: wh[b, n, h, tg*8+m]
    # = w[b, t=tg*16+2m+h, n]
    k = np.arange(C)
    whost = np.empty((32, NB, 2, C), np.float32)
    for h in range(2):
        tidx = (k // 8) * 16 + (k % 8) * 2 + h
        whost[:, :, h, :] = w[:, tidx, :].transpose(0, 2, 1)

    nc = _get_program()
    in_maps = []
    for c in range(NCORES):
        bs = slice(B4 * c, B4 * (c + 1))
        wbig_np = np.empty((2, C, B4, NB, C), np.float32)
        for h in range(2):
            wbig_np[h] = np.broadcast_to(
                whost[bs][:, :, h, :][None], (C, B4, NB, C)
            )
        in_maps.append(
            dict(
                ukb=np.ascontiguousarray(ukb[bs]),
                xkb=np.ascontiguousarray(xkb[bs]),
                xo=np.ascontiguousarray(x[bs]),
                wbig=np.ascontiguousarray(wbig_np.reshape(T, B4 * NB * C)),
                md=np.ascontiguousarray(mdv[bs]),
                mb8=mb8_np,
                i2=i2_np,
                wemb=wemb_np,
                ones64=ones_np,
                id128=id128_np,
            )
        )

    from concourse.bass_utils import run_bass_kernel_spmd

    res = run_bass_kernel_spmd(nc, in_maps, core_ids=list(range(NCORES)))
    _CACHE["last_results"] = res
    P = np.concatenate([r["pfunc"] for r in res.results], axis=0)
    E = np.concatenate([r["emb"] for r in res.results], axis=0)
    return P, E


# revision 37
# speedup vs baseline: 162.2780x; 162.2780x over previous
"""Trainium2 Bass kernel for nn_BandFunctionalPrior.

Math reduction: e = x*W_e + b_e is affine in the scalar x, so the
attention scores collapse to rank-1 plus j-constant terms that cancel
in softmax:

    s[t,n,i,j] = u_i * x_j  (+ j-const, cancelled)
    u = (alpha_n * x + gamma_n) / sqrt(d_s)

Per (batch, t, band): P = row-softmax(u x^T + maskbias), output
P_func = sum_n w_n P_n plus a tiny embeddings path.  Pure data
parallel over batch (4 per core, 8 cores).

Device geometry: for each (b, band), 128 t-values form 8 tile groups
of 16; a tile group's 16 stacked 64x64 score matrices are computed by
ONE K=112 bf16 matmul into a [128, 512] PSUM slice:
  rows  0..16: uh pair-diagonal (block h-diag of bf16-high u)
  rows 16..32: ul pair-diagonal (bf16 residual of u)
  rows 32..48: uh pair-diagonal again
  rows 48..112: [I64|I64] selection rows (constant)
against rhs rows (xh-diag, xh-diag, xl-diag, mb8) — i.e. the split
product uh*xh + ul*xh + uh*xl (abs err ~|s|*2^-18) with the adjacency
mask bias added via the constant rows.  ACT exponentiates 4 tiles at a
time ([128, 2048] PSUM -> bf16 SBUF); DVE does row sums + reciprocal +
weighting + the final fp32 combine; GPSIMD expands the row scales and
does the intermediate band accumulations.
"""

import sys

for _p in ("/opt/trn_rl_repo",):
    if _p not in sys.path:
        sys.path.insert(0, _p)

import ml_dtypes
import numpy as np

BF16 = np.dtype(ml_dtypes.bfloat16)

NCORES = 8
B4 = 4          # batches per core
NB = 5          # bands
C = 64          # electrodes
T = 128         # time steps
TG = 8          # tile groups per (b, band); 16 t's each
NSB = 2         # supertiles per (b, band); 4 tile groups each
NEG = -30000.0  # mask bias; exp(s + NEG) == 0 in fp32

_CACHE = {}


def _build_program():
    import concourse.bacc as bacc
    import concourse.mybir as mybir
    from concourse.tile import TileContext

    fp32 = mybir.dt.float32
    bf16 = mybir.dt.bfloat16
    ALU = mybir.AluOpType
    ACTF = mybir.ActivationFunctionType
    AXIS = mybir.AxisListType

    nc = bacc.Bacc(None, target_bir_lowering=False)

    ukb = nc.dram_tensor("ukb", (B4, NB, 48, TG * 128), bf16, kind="ExternalInput")
    xkb = nc.dram_tensor("xkb", (B4, NB, 48, TG * 512), bf16, kind="ExternalInput")
    xo = nc.dram_tensor("xo", (B4, NB, C, T), fp32, kind="ExternalInput")
    wbig = nc.dram_tensor("wbig", (T, B4 * NB * C), fp32, kind="ExternalInput")
    md = nc.dram_tensor("md", (B4, T, NB), fp32, kind="ExternalInput")
    mb8 = nc.dram_tensor("mb8", (C, TG * C), bf16, kind="ExternalInput")
    i2 = nc.dram_tensor("i2", (C, 2 * C), bf16, kind="ExternalInput")
    wemb = nc.dram_tensor("wemb", (2 * NB, C), fp32, kind="ExternalInput")
    ones64 = nc.dram_tensor("ones64", (C, 1), fp32, kind="ExternalInput")
    id128 = nc.dram_tensor("id128", (T, T), fp32, kind="ExternalInput")
    # device-native layout: (b, h, i, tg, m, j); host reorders to (b,t,i,j)
    pfunc = nc.dram_tensor(
        "pfunc", (B4, 2, C, TG, 8, C), fp32, kind="ExternalOutput"
    )
    emb = nc.dram_tensor("emb", (B4, T, C), fp32, kind="ExternalOutput")

    SB = 4 * 512          # supertile free width (4 tile groups)
    KK = 112              # matmul contraction: 16*3 pair rows + 64 mask rows

    with TileContext(nc) as tc:
        # ---------------- embeddings (tiny, own psum scope) ----------------
        with (
            tc.tile_pool(name="econst", bufs=1) as ecp,
            tc.tile_pool(name="ework", bufs=4) as ewp,
            tc.tile_pool(name="epsum", bufs=2, space="PSUM") as epp,
        ):
            wemb_sb = ecp.tile([2 * NB, C], fp32)
            nc.sync.dma_start(wemb_sb, wemb[:, :])
            ones_sb = ecp.tile([C, 1], fp32)
            nc.sync.dma_start(ones_sb, ones64[:, :])
            id128_sb = ecp.tile([T, T], fp32)
            nc.sync.dma_start(id128_sb, id128[:, :])
            for b in range(B4):
                xbars = epp.tile([T, NB], fp32, tag="xbars")
                for n in range(NB):
                    xo_sb = ewp.tile([C, T], fp32, tag="xo")
                    nc.sync.dma_start(xo_sb, xo[b, n])
                    nc.tensor.matmul(
                        xbars[:, n : n + 1], xo_sb, ones_sb, start=True, stop=True
                    )
                mdv_sb = ewp.tile([T, NB], fp32, tag="mdv")
                nc.sync.dma_start(mdv_sb, md[b])
                coef = ewp.tile([T, 2 * NB], fp32, tag="coef")
                nc.vector.tensor_mul(coef[:, :NB], xbars[:, :], mdv_sb[:, :])
                nc.vector.tensor_copy(coef[:, NB:], mdv_sb[:, :])
                coefT_ps = epp.tile([2 * NB, T], fp32, tag="coefT")
                nc.tensor.transpose(coefT_ps, coef, id128_sb)
                coefT_sb = ewp.tile([2 * NB, T], fp32, tag="coefTs")
                nc.vector.tensor_copy(coefT_sb, coefT_ps)
                emb_ps = epp.tile([T, C], fp32, tag="embp")
                nc.tensor.matmul(emb_ps, coefT_sb, wemb_sb, start=True, stop=True)
                emb_sb = ewp.tile([T, C], fp32, tag="embs")
                nc.vector.tensor_copy(emb_sb, emb_ps)
                nc.sync.dma_start(emb[b], emb_sb)

        # ---------------- attention ----------------
        with (
            tc.tile_pool(name="banks", bufs=1) as bankp,
            tc.tile_pool(name="fwork", bufs=3) as fworkp,
            tc.tile_pool(name="paccp", bufs=2) as paccp,
            tc.tile_pool(name="pout", bufs=2) as poutp,
            tc.tile_pool(name="small", bufs=8) as smallp,
            tc.tile_pool(name="psum", bufs=2, space="PSUM") as psump,
        ):
            # persistent banks (ping-pong x2): const mask rows written once,
            # u/x rows DMA'd per (b, n) from host-prebuilt images
            kbanks, xkbanks = [], []
            for kk in range(2):
                kbk = bankp.tile([KK, TG * 128], bf16, name=f"kbank{kk}")
                for tgb in range(TG):
                    i2_dst = kbk[48:, tgb * 128 : (tgb + 1) * 128]
                    nc.sync.dma_start(i2_dst, i2[:, :])
                kbanks.append(kbk)
                xbk = bankp.tile([KK, TG * 512], bf16, name=f"xkbank{kk}")
                for tgb in range(TG):
                    mb_dst = xbk[48:, tgb * 512 : (tgb + 1) * 512]
                    nc.sync.dma_start(mb_dst, mb8[:, :])
                xkbanks.append(xbk)
            # full w table, loaded once: [128=(h,i), (b, n, tg, m)]
            wt = bankp.tile([T, B4 * NB * C], fp32, name="wtall")
            nc.sync.dma_start(wt, wbig[:, :])

            for b in range(B4):
                pacc = [
                    paccp.tile([128, 2 * SB], bf16, tag="pacc", name=f"pacc_{b}")
                ]
                for n in range(NB):
                    kk = (b * NB + n) % 2
                    kbk, xbk = kbanks[kk], xkbanks[kk]
                    # fill u/x rows from host-prebuilt images: 2 big DMAs
                    nc.sync.dma_start(kbk[:48, :], ukb[b, n])
                    nc.sync.dma_start(xbk[:48, :], xkb[b, n])
                    wcol0 = (b * NB + n) * C

                    F = fworkp.tile([128, 2 * SB], bf16, tag="F")
                    for half in range(2):
                        ps = psump.tile([128, SB], fp32, tag="ps")
                        for q2 in range(4):
                            tg = half * 4 + q2
                            nc.tensor.matmul(
                                ps[:, q2 * 512 : (q2 + 1) * 512],
                                kbk[:, tg * 128 : (tg + 1) * 128],
                                xbk[:, tg * 512 : (tg + 1) * 512],
                                start=True,
                                stop=True,
                            )
                        nc.scalar.activation(
                            F[:, half * SB : (half + 1) * SB], ps, ACTF.Exp
                        )
                    Fv = F.rearrange("p (m j) -> p m j", j=C)
                    Z = smallp.tile([128, 8 * TG], fp32, tag="Z")
                    nc.vector.tensor_reduce(Z, Fv, axis=AXIS.X, op=ALU.add)
                    rr = smallp.tile([128, 8 * TG], fp32, tag="rr")
                    nc.vector.reciprocal(rr, Z)
                    r2 = smallp.tile([128, 8 * TG], bf16, tag="r2")
                    nc.vector.tensor_mul(r2, rr, wt[:, wcol0 : wcol0 + C])
                    rb = r2[:, :, None].to_broadcast((128, 8 * TG, C))
                    if n == 0:
                        nc.vector.tensor_mul(
                            pacc[0].rearrange("p (m j) -> p m j", j=C), rb, Fv)
                    elif n < NB - 1:
                        tmp = fworkp.tile([128, 2 * SB], bf16, tag="tmp")
                        nc.vector.tensor_mul(
                            tmp.rearrange("p (m j) -> p m j", j=C), rb, Fv)
                        nc.vector.tensor_add(pacc[0], pacc[0], tmp)
                    else:
                        tmp = fworkp.tile([128, 2 * SB], bf16, tag="tmp")
                        nc.vector.tensor_mul(
                            tmp.rearrange("p (m j) -> p m j", j=C), rb, Fv)
                        po = poutp.tile([128, 2 * SB], fp32, tag="po")
                        nc.vector.tensor_add(po, pacc[0], tmp)
                        # contiguous store in device-native layout
                        for h in range(2):
                            sview = po[h * C : (h + 1) * C, :].rearrange(
                                "i (tg m j) -> i tg m j", tg=TG, j=C
                            )
                            nc.sync.dma_start(pfunc[b, h], sview)
    nc.finalize()
    return nc


def _get_program():
    if "nc" not in _CACHE:
        _CACHE["nc"] = _build_program()
    return _CACHE["nc"]


def _split_bf16(a):
    hi = a.astype(BF16)
    lo = (a - hi.astype(np.float32)).astype(BF16)
    return hi, lo


def kernel(x, adj, active_mask, active_alpha, W_e, b_e, Wq, Wk):
    x = np.asarray(x, np.float32)
    adj = np.asarray(adj, np.float32)
    active_mask = np.asarray(active_mask, np.float32)
    active_alpha = np.asarray(active_alpha, np.float32)
    W_e = np.asarray(W_e, np.float32)
    b_e = np.asarray(b_e, np.float32)
    Wq = np.asarray(Wq, np.float32)
    Wk = np.asarray(Wk, np.float32)

    ds = Wq.shape[0]
    Aq = W_e @ Wq.T
    Ak = W_e @ Wk.T
    Bq = b_e @ Wq.T
    a = (Aq * Ak).sum(-1) / np.sqrt(ds)
    g = (Bq * Ak).sum(-1) / np.sqrt(ds)

    xt = np.ascontiguousarray(x.transpose(0, 1, 3, 2))          # (32, 5, 128, 64)
    utf = (a[None, :, None, None] * xt + g[None, :, None, None]).astype(np.float32)
    xh, xl = _split_bf16(xt)
    uh, ul = _split_bf16(utf)

    # bank images; rows (g, m, h) = 16g + 2m + h
    # ukb cols (tg, c2, i): u_g(t=tg*16+2m+h) at c2 == h
    # xkb cols (tg, mc, j): x_g(t) at mc == m
    ukb = np.zeros((32, NB, 48, TG * 128), BF16)
    xkb = np.zeros((32, NB, 48, TG * 512), BF16)
    ukb_v = ukb.reshape(32, NB, 3, 8, 2, TG, 2, C)
    xkb_v = xkb.reshape(32, NB, 3, 8, 2, TG, 8, C)
    for gi, us in ((0, uh), (1, ul), (2, uh)):
        uu = us.reshape(32, NB, TG, 8, 2, C)                     # (b,n,tg,m,h,i)
        for h in range(2):
            ukb_v[:, :, gi, :, h, :, h, :] = uu[:, :, :, :, h, :].transpose(
                0, 1, 3, 2, 4
            )
    for gi, xs in ((0, xh), (1, xh), (2, xl)):
        xx = xs.reshape(32, NB, TG, 8, 2, C)
        for h in range(2):
            for m in range(8):
                xkb_v[:, :, gi, m, h, :, m, :] = xx[:, :, :, m, h, :]

    w = (active_alpha * active_mask).astype(np.float32)          # (32, 128, 5)
    cnt = np.clip(active_mask.sum(-1, keepdims=True), 1.0, None)
    mdv = (active_mask / cnt).astype(np.float32)                 # (32, 128, 5)
    mb = np.where(adj > 0, 0.0, NEG).astype(np.float32)
    mb8_np = np.ascontiguousarray(np.tile(mb, (1, TG))).astype(BF16)
    i2_np = np.ascontiguousarray(
        np.concatenate([np.eye(C, dtype=np.float32)] * 2, axis=1)
    ).astype(BF16)
    wemb_np = np.ascontiguousarray(
        np.concatenate([W_e / float(C), b_e], axis=0)
    ).astype(np.float32)
    ones_np = np.ones((C, 1), np.float32)
    id128_np = np.eye(T, dtype=np.float32)

    # w rearranged for the (h, i)-partition tiles: wh[b, n, h, tg*8+m]
    # = w[b, t=tg*16+2m+h, n]
    k = np.arange(C)
    whost = np.empty((32, NB, 2, C), np.float32)
    for h in range(2):
        tidx = (k // 8) * 16 + (k % 8) * 2 + h
        whost[:, :, h, :] = w[:, tidx, :].transpose(0, 2, 1)

    nc = _get_program()
    in_maps = []
    for c in range(NCORES):
        bs = slice(B4 * c, B4 * (c + 1))
        wbig_np = np.empty((2, C, B4, NB, C), np.float32)
        for h in range(2):
            wbig_np[h] = np.broadcast_to(
                whost[bs][:, :, h, :][None], (C, B4, NB, C)
            )
        in_maps.append(
            dict(
                ukb=np.ascontiguousarray(ukb[bs]),
                xkb=np.ascontiguousarray(xkb[bs]),
                xo=np.ascontiguousarray(x[bs]),
                wbig=np.ascontiguousarray(wbig_np.reshape(T, B4 * NB * C)),
                md=np.ascontiguousarray(mdv[bs]),
                mb8=mb8_np,
                i2=i2_np,
                wemb=wemb_np,
                ones64=ones_np,
                id128=id128_np,
            )
        )

    from concourse.bass_utils import run_bass_kernel_spmd

    res = run_bass_kernel_spmd(nc, in_maps, core_ids=list(range(NCORES)))
    _CACHE["last_results"] = res
    # (b, h, i, tg, m, j) -> (b, t=tg*16+2m+h, i, j)
    Pd = np.concatenate([r["pfunc"] for r in res.results], axis=0)
    P = np.ascontiguousarray(
        Pd.transpose(0, 3, 4, 1, 2, 5).reshape(32, T, C, C)
    )
    E = np.concatenate([r["emb"] for r in res.results], axis=0)
    return P, E
